# revision 20
# baseline (speedup 1.0000x reference)
"""DCNv2 deformable RoI pooling on 8 Trainium2 NeuronCores.

Strategy (roi-sharded, host-packed windows + bf16 matmul reduce):
  - Host: replicate the reference's f32 sampling math from (rois, offset)
    (tiny tensors), derive for each roi its bbox window [hs, ws] on the
    feature map and a dense weight matrix Wmat[px, 49] folding bilinear
    weights, validity and 1/count:
        out[n, c, bin] = sum_px Fwin[px, c] * Wmat[px, bin].
  - Rois are sorted by window pixel count and dealt round-robin to the 8
    cores so slot s has identical (compile-time) shapes on every core.
  - Host packs, per (core, slot), the window pixels AND the weight rows
    into ONE contiguous bf16 buffer laid out partition-major:
    partition p holds G pixels (G*256 feature values, then G*49 weights).
    All device DMAs are therefore static + contiguous with multi-KB
    per-partition runs; big slots are issued on the gpsimd SWDGE queue
    (descriptors spread over all 16 SDMA engines), small slots on the two
    HWDGE queues (sync/scalar).
  - Device per slot (1 roi): one DMA, then per (half, g):
    matmul(psum[128, 49], win[0:K, g*C+half*128 :+128], wm[0:K, g*49 :+49])
    accumulating over g; psum -> SBUF (bf16), one output DMA at the end.
  - bf16 is safe: the harness gate is rel_err < 2e-2, bf16 lands ~3e-3.
"""
import sys

sys.path.insert(0, "/opt/trn_rl_repo")

import numpy as np
import ml_dtypes

SPATIAL_SCALE = 0.0625
POOLED = 7
SAMPLE = 4
TRANS_STD = 0.1
B, C, H, W = 2, 256, 160, 160
N_ROIS = 128
NCORES = 8
RPB = N_ROIS // NCORES  # rois per core (= slots)
P, S = POOLED, SAMPLE
NBINS = P * P
CH = C // 2  # stationary half width
COLW = C + NBINS  # per-pixel packed row: 256 features + 49 weights
G_FLOOR = 4  # min pixels per partition -> 2440B runs (mergeable to <8KB packets)

f32 = np.float32
bf16 = ml_dtypes.bfloat16


# ----------------------------------------------------------------- host plan

def _sample_math(rois, offset):
    rois = rois.astype(f32)
    offset = offset.astype(f32)
    b = rois[:, 0].astype(np.int32)
    x1, y1, x2, y2 = rois[:, 1], rois[:, 2], rois[:, 3], rois[:, 4]
    rsw = (np.round(x1) * f32(SPATIAL_SCALE) - f32(0.5)).astype(f32)
    rsh = (np.round(y1) * f32(SPATIAL_SCALE) - f32(0.5)).astype(f32)
    rew = ((np.round(x2) + f32(1.0)) * f32(SPATIAL_SCALE) - f32(0.5)).astype(f32)
    reh = ((np.round(y2) + f32(1.0)) * f32(SPATIAL_SCALE) - f32(0.5)).astype(f32)
    rw = np.maximum(rew - rsw, f32(0.1))
    rh = np.maximum(reh - rsh, f32(0.1))
    bw, bh = (rw / P).astype(f32), (rh / P).astype(f32)
    sw, sh = (bw / S).astype(f32), (bh / S).astype(f32)
    tx = offset[:, 0] * f32(TRANS_STD)
    ty = offset[:, 1] * f32(TRANS_STD)
    pw_i = np.arange(P, dtype=f32)
    ph_i = np.arange(P, dtype=f32)
    wstart = (pw_i[None, None, :] * bw[:, None, None] + rsw[:, None, None]
              + tx * rw[:, None, None]).astype(f32)
    hstart = (ph_i[None, :, None] * bh[:, None, None] + rsh[:, None, None]
              + ty * rh[:, None, None]).astype(f32)
    iw = np.arange(S, dtype=f32)
    x = (wstart[..., None] + iw * sw[:, None, None, None]).astype(f32)
    y = (hstart[..., None] + iw * sh[:, None, None, None]).astype(f32)
    validx = (x >= -0.5) & (x <= W - 0.5)
    validy = (y >= -0.5) & (y <= H - 0.5)
    xc = np.clip(x, f32(0.0), f32(W - 1.0))
    yc = np.clip(y, f32(0.0), f32(H - 1.0))
    x0 = np.floor(xc); x1c = np.ceil(xc)
    y0 = np.floor(yc); y1c = np.ceil(yc)
    dx = (xc - x0).astype(f32)
    dy = (yc - y0).astype(f32)
    cnt = (validx.sum(-1) * validy.sum(-1)).astype(f32)
    denom = np.maximum(cnt, f32(1.0))
    return dict(b=b, validx=validx, validy=validy,
                x0=x0.astype(np.int32), x1=x1c.astype(np.int32),
                y0=y0.astype(np.int32), y1=y1c.astype(np.int32),
                dx=dx, dy=dy, denom=denom)


def _plan(rois, offset):
    sm = _sample_math(rois, offset)
    nroi = sm["b"].shape[0]
    xmin = np.zeros(nroi, np.int64); xmax = np.zeros(nroi, np.int64)
    ymin = np.zeros(nroi, np.int64); ymax = np.zeros(nroi, np.int64)
    vx, vy = sm["validx"], sm["validy"]
    for n in range(nroi):
        joint = (vx[n].any(-1) & vy[n].any(-1))
        if not joint.any():
            continue
        selx = vx[n] & joint[..., None]
        sely = vy[n] & joint[..., None]
        xmin[n] = sm["x0"][n][selx].min(); xmax[n] = sm["x1"][n][selx].max()
        ymin[n] = sm["y0"][n][sely].min(); ymax[n] = sm["y1"][n][sely].max()
    w_need = xmax - xmin + 1
    h_need = ymax - ymin + 1
    px_need = w_need * h_need

    order = np.argsort(-px_need, kind="stable")
    slot_K = []; slot_G = []
    for s in range(RPB):
        grp = order[s * NCORES:(s + 1) * NCORES]
        pxmax = int(px_need[grp].max())
        G = max(G_FLOOR, -(-pxmax // 128))
        K = -(-pxmax // G)
        slot_K.append(K); slot_G.append(G)

    # dense per-roi weight over its bbox, flattened row-major (h, w)
    wmats = {}
    for n in range(nroi):
        s = int(np.where(order == n)[0][0]) // NCORES
        hs, ws = int(h_need[n]), int(w_need[n])
        Ay = np.zeros((NBINS, hs), f32)
        Bx = np.zeros((NBINS, ws), f32)
        vxn = sm["validx"][n].reshape(NBINS, S)
        vyn = sm["validy"][n].reshape(NBINS, S)
        x0 = sm["x0"][n].reshape(NBINS, S) - xmin[n]
        x1 = sm["x1"][n].reshape(NBINS, S) - xmin[n]
        y0 = sm["y0"][n].reshape(NBINS, S) - ymin[n]
        y1 = sm["y1"][n].reshape(NBINS, S) - ymin[n]
        dx = sm["dx"][n].reshape(NBINS, S)
        dy = sm["dy"][n].reshape(NBINS, S)
        bins = np.repeat(np.arange(NBINS), S)
        np.add.at(Bx, (bins, np.clip(x0, 0, ws - 1).ravel()), ((1 - dx) * vxn).ravel())
        np.add.at(Bx, (bins, np.clip(x1, 0, ws - 1).ravel()), (dx * vxn).ravel())
        np.add.at(Ay, (bins, np.clip(y0, 0, hs - 1).ravel()), ((1 - dy) * vyn).ravel())
        np.add.at(Ay, (bins, np.clip(y1, 0, hs - 1).ravel()), (dy * vyn).ravel())
        Wpx = Ay[:, :, None] * Bx[:, None, :] / sm["denom"][n].reshape(NBINS, 1, 1)
        wmats[n] = Wpx.reshape(NBINS, hs * ws).T.astype(f32)  # [px, 49]

    return dict(sm=sm, order=order, slot_K=slot_K, slot_G=slot_G,
                xmin=xmin, ymin=ymin, w_need=w_need, h_need=h_need,
                wmats=wmats)


# --------------------------------------------------------------- bass program

_PROGRAM_CACHE = {}

# PE consumption order: 8 big gpsimd slots while they stream in, then the
# HWDGE-delivered slots 8..15 in expected arrival order.
_SLOT_ORDER = list(range(RPB))
# one DMA per slot; gpsimd SWDGE carries exactly 8 slots (the ring blocks
# the 9th in-flight DMA), the two HWDGE queues carry 4 mid/small slots each
# (~0.6MB @ ~28GB/s), arriving just ahead of the PE tail.
_PAIRS = ([(s,) for s in range(8)] + [(9,), (11,), (13,), (15,)]
          + [(8,), (10,), (12,), (14,)])
_PAIR_Q = ["gpsimd"] * 8 + ["sync"] * 4 + ["scalar"] * 4
# output chunk boundaries (PE positions): final chunk is a single position
# so the post-matmul output tail is minimal.
_OUT_AFTER = {3: 0, 7: 4, 11: 8, 14: 12, 15: 15}


def _pair_layout(slot_K, slot_G):
    """Shared host/device layout: per pair (Kmx, total cols, per-slot col
    offsets, flat element offset into wbuf)."""
    cols = [slot_G[s] * COLW for s in range(RPB)]
    info = []
    off = 0
    slot_loc = {}
    for i, p in enumerate(_PAIRS):
        kmx = max(slot_K[s] for s in p)
        cwsum = sum(cols[s] for s in p)
        co = 0
        for s in p:
            slot_loc[s] = (i, co)
            co += cols[s]
        info.append((kmx, cwsum, off))
        off += kmx * cwsum
    return cols, info, slot_loc, off


def _build_program(slot_K, slot_G):
    import concourse.bass as bass
    import concourse.bacc as bacc
    import concourse.mybir as mybir
    import concourse.tile as tile

    cols, pinfo, slot_loc, tot = _pair_layout(slot_K, slot_G)
    ocols = RPB * 2 * NBINS

    nc = bacc.Bacc("TRN2", target_bir_lowering=False, debug=False,
                   num_devices=NCORES)
    wbuf = nc.declare_dram_parameter("wbuf", [tot], mybir.dt.bfloat16,
                                     isOutput=False)
    out = nc.declare_dram_parameter("out", [128, ocols], mybir.dt.bfloat16,
                                    isOutput=True)

    with tile.TileContext(nc) as tc:
        with (
            tc.tile_pool(name="small", bufs=1) as small,
            tc.tile_pool(name="winp", bufs=1) as winp,
            tc.tile_pool(name="psum", bufs=8, space="PSUM") as psump,
        ):
            ostage = small.tile([128, ocols], mybir.dt.bfloat16)
            queues = {"gpsimd": nc.gpsimd, "sync": nc.sync, "scalar": nc.scalar}

            ptiles = []
            for i, p in enumerate(_PAIRS):
                kmx, cwsum, off = pinfo[i]
                win = winp.tile([128, cwsum], mybir.dt.bfloat16, tag=f"w{i}")
                ptiles.append(win)
                src = bass.AP(wbuf[:].tensor, int(off), [[cwsum, kmx], [1, cwsum]])
                queues[_PAIR_Q[i]].dma_start(win[0:kmx, 0:cwsum], src)

            for pos, s in enumerate(_SLOT_ORDER):
                K, G = slot_K[s], slot_G[s]
                pi, co = slot_loc[s]
                win = ptiles[pi]
                for half in range(2):
                    pt = psump.tile([128, NBINS], mybir.dt.float32, tag="pt")
                    for g in range(G):
                        nc.tensor.matmul(
                            pt[:, :],
                            win[0:K, co + g * C + half * CH:
                                co + g * C + half * CH + CH],
                            win[0:K, co + G * C + g * NBINS:
                                co + G * C + (g + 1) * NBINS],
                            start=(g == 0), stop=(g == G - 1),
                        )
                    nc.vector.tensor_copy(
                        ostage[:, (pos * 2 + half) * NBINS:
                               (pos * 2 + half + 1) * NBINS],
                        pt[:, :])
                if pos in _OUT_AFTER:
                    c0 = _OUT_AFTER[pos] * 2 * NBINS
                    c1 = (pos + 1) * 2 * NBINS
                    osrc = ostage[:, c0:c1]
                    odst = bass.AP(out[:].tensor, c0,
                                   [[ocols, 128], [1, c1 - c0]])
                    nc.gpsimd.dma_start(odst, osrc)

    nc.compile()
    return nc


# -------------------------------------------------------------------- kernel

TRACE = False
LAST_RESULTS = None


def kernel(input, rois, offset):
    from concourse.bass_utils import run_bass_kernel_spmd

    input = np.ascontiguousarray(np.asarray(input, f32))
    rois = np.asarray(rois, f32)
    offset = np.asarray(offset, f32)

    pl = _plan(rois, offset)
    slot_K, slot_G = pl["slot_K"], pl["slot_G"]
    order = pl["order"]
    sm = pl["sm"]

    nhwc = np.transpose(input, (0, 2, 3, 1)).astype(bf16)  # [B, H, W, C]

    cols, pinfo, slot_loc, tot = _pair_layout(slot_K, slot_G)

    in_maps = []
    for c in range(NCORES):
        buf = np.zeros(tot, bf16)
        for i, p in enumerate(_PAIRS):
            kmx, cwsum, off = pinfo[i]
            blk = np.zeros((kmx, cwsum), bf16)
            for s in p:
                K, G = slot_K[s], slot_G[s]
                _, co = slot_loc[s]
                L = K * G
                n = int(order[s * NCORES + c])
                hs, ws = int(pl["h_need"][n]), int(pl["w_need"][n])
                px = hs * ws
                bI, bY, bX = (int(sm["b"][n]), int(pl["ymin"][n]),
                              int(pl["xmin"][n]))
                winpix = np.zeros((L, C), bf16)
                winpix[:px] = nhwc[bI, bY:bY + hs, bX:bX + ws].reshape(px, C)
                wmr = np.zeros((L, NBINS), bf16)
                wmr[:px] = pl["wmats"][n].astype(bf16)
                blk[:K, co:co + G * C] = winpix.reshape(K, G * C)
                blk[:K, co + G * C:co + cols[s]] = wmr.reshape(K, G * NBINS)
            buf[off:off + kmx * cwsum] = blk.reshape(-1)
        in_maps.append({"wbuf": buf})

    key = (tuple(slot_K), tuple(slot_G))
    if key not in _PROGRAM_CACHE:
        _PROGRAM_CACHE[key] = _build_program(slot_K, slot_G)
    nc = _PROGRAM_CACHE[key]

    kwargs = {}
    if TRACE:
        kwargs = dict(trace=True, trace_cores=list(range(NCORES)))
    res = run_bass_kernel_spmd(nc, in_maps, list(range(NCORES)), **kwargs)
    global LAST_RESULTS
    LAST_RESULTS = res

    out_full = np.zeros((N_ROIS, C, NBINS), f32)
    for c in range(NCORES):
        o = np.asarray(res.results[c]["out"]).astype(f32)  # [128, RPB*2*49]
        for pos, s in enumerate(_SLOT_ORDER):
            n = int(order[s * NCORES + c])
            out_full[n, 0:CH] = o[:, (pos * 2) * NBINS:(pos * 2 + 1) * NBINS]
            out_full[n, CH:C] = o[:, (pos * 2 + 1) * NBINS:(pos * 2 + 2) * NBINS]
    return out_full.reshape(N_ROIS, C, P, P)
